# revision 30
# baseline (speedup 1.0000x reference)
"""MultiHeadContrastive loss on 8 TRN2 NeuronCores (Bass/Tile SPMD).

v2: symmetric-exp scheme.  exp(sim) is symmetric, so each (i,j) pair is
exponentiated ONCE globally.  Rows are permuted host-side so background
rows (label==0) form a global prefix (all inside block 0).  Core c
computes:
  - its diagonal block (own rows x own cols),
  - full pair-blocks {c, c+1}, {c, c+2}, {c, c+3} (own rows x partner
    cols, 1024 wide), stored at slots rotated so a block-0-column tile
    (cores 5,6,7) always sits at slot 0,
  - the {c, c+4} pair split into two quarter-tiles [512 rows x 512
    cols] (diagonal quadrants for c<4, anti-diagonal for c>=4) so the
    pair is covered exactly once globally with a uniform program.

Row sums of exp come from the scalar engine's accumulator (accum_out on
the exp activation itself).  Column sums (the transposed halves, owed
to partner rows) come from zero-padded-lhsT matmuls on the PE streaming
the bf16 exp tiles (emitted one tile behind the sims so the PE never
stalls waiting for an exp), accumulated in PSUM, transposed back to
partition-major panels, and exchanged with two small AllGathers (fg
panels overlap the cls j-loop; cls panels overlap the cls diagonal pass
which runs last).  Background-prefix subsums use a bg-indicator lhsT
column for the colsums and small DVE subrange reduces for own rows,
gated by per-core flag inputs so the instruction stream stays uniform.
"""
import numpy as np
import ml_dtypes

import concourse.bacc as bacc
import concourse.mybir as mybir
import concourse.tile as tile
import concourse.bass_utils as bass_utils
from concourse.tile_rust import add_dep_helper

NCORES = 8
N, C, H, DF, DC = 8192, 1024, 256, 64, 128
SH = N // NCORES      # 1024 rows per core
NIC = SH // 128       # 8 i-chunks of 128 rows
NCLS = 21
EPS = 1e-8
TAU = 0.2
KW = 4 * SH           # remote z panel width: 3 full + 2 half slots

BF16 = mybir.dt.bfloat16
F32 = mybir.dt.float32
F8 = mybir.dt.float8e4
AF = mybir.ActivationFunctionType
ALU = mybir.AluOpType

_cached = {}


def _build(n_bg):
    assert 0 <= n_bg < 512, n_bg
    NBGF, NBGR = n_bg // 128, n_bg % 128

    nc = bacc.Bacc("TRN2", target_bir_lowering=False, debug=False,
                   num_devices=NCORES)

    def inp(name, shape, dt):
        return nc.dram_tensor(name, shape, dt, kind="ExternalInput")

    xT = inp("xT", [C, SH], BF16)            # own rows, transposed
    w1f = inp("w1f", [C, H], BF16)
    w1c = inp("w1c", [C, H], BF16)
    b1f = inp("b1f", [128, H // 128], F32)   # partition-major
    b1c = inp("b1c", [128, H // 128], F32)
    w2f = inp("w2f", [H, DF], BF16)
    w2c = inp("w2c", [H, DC], BF16)
    b2f8 = inp("b2f8", [128, NIC * DF], F32)   # b2 bcast, tiled per i-chunk
    b2c8 = inp("b2c8", [128, NIC * DC], F32)
    fgown = inp("fgown", [128, NIC], F32)    # own fg mask
    iou = inp("iou", [128, NIC], F32)        # own ious
    ohb = inp("ohb", [128, NIC * NCLS], BF16)  # own-label one-hot per i-chunk
    ident = inp("ident", [128, 128], BF16)
    identF = inp("identF", [128, 128], F32)
    # fg colsum lhsT variants [128, 4*8]: block k nonzero col:
    #   cswt: col 2k = ones (tot); cswb: col 2k+1 = 1 on core 0 (bg via
    #   esum snapshot); cswp: col 2k+1 = partial-chunk bg indicator
    cswt = inp("cswt", [128, 4 * 8], BF16)
    cswb = inp("cswb", [128, 4 * 8], BF16)
    cswp = inp("cswp", [128, 4 * 8], BF16)
    cswc = inp("cswc", [128, 4 * 4], BF16)   # cls: col k of block k = ones
    # bg gates [128, 3*NIC] g-major: slot0-tile | k4-hi | diag
    bgw = inp("bgw", [128, 3 * NIC], F32)

    psums = nc.dram_tensor("psums", [1, 8], F32, kind="ExternalOutput")

    # collective buffers (z in fp8; cls payload carries the raw f32
    # class-sum bytes as extra fp8 rows)
    CBW = (DC + 1) * 4                              # 516 fp8 per cb row
    CROWS = DC + NCLS                               # 149
    zpackF = nc.dram_tensor("zpackF", [DF, SH], F8)
    zgathF = nc.dram_tensor("zgathF", [NCORES * DF, SH], F8,
                            addr_space="Shared")
    zpackC = nc.dram_tensor("zpackC", [CROWS, SH], F8)
    zgathC = nc.dram_tensor("zgathC", [NCORES * CROWS, SH], F8,
                            addr_space="Shared")
    oboxF = nc.dram_tensor("oboxF", [128, 64], F32)
    goboxF = nc.dram_tensor("goboxF", [NCORES * 128, 64], F32,
                            addr_space="Shared")
    oboxC = nc.dram_tensor("oboxC", [128, 32], F32)
    goboxC = nc.dram_tensor("goboxC", [NCORES * 128, 32], F32,
                            addr_space="Shared")

    rg = [list(range(NCORES))]

    with tile.TileContext(nc) as tc:
        with (
            tc.tile_pool(name="persist", bufs=1) as P,
            tc.tile_pool(name="work", bufs=2) as W,
            tc.tile_pool(name="exps", bufs=4) as EX,
        ):
            nc.scalar.add_instruction(mybir.InstLoadActFuncSet(
                name="actset_ln_exp", ins=[], outs=[], act_func_set_id=6))

            # ---- load persistent inputs into SBUF ----
            xT_sb = P.tile([128, (C // 128) * SH], BF16, tag="xT")
            xT_r = xT.ap().rearrange("(c p) r -> p c r", p=128)
            w1f_sb = P.tile([128, (C // 128) * H], BF16, tag="w1f")
            w1f_r = w1f.ap().rearrange("(c p) h -> p c h", p=128)
            w1c_sb = P.tile([128, (C // 128) * H], BF16, tag="w1c")
            w1c_r = w1c.ap().rearrange("(c p) h -> p c h", p=128)
            for c in range(C // 128):
                nc.sync.dma_start(w1f_sb[:, c * H:(c + 1) * H],
                                  w1f_r[:, c:c + 1, :])
                nc.sync.dma_start(xT_sb[:, c * SH:(c + 1) * SH],
                                  xT_r[:, c:c + 1, :])
            b1f_sb = P.tile([128, H // 128], F32, tag="b1f")
            nc.sync.dma_start(b1f_sb[:, :], b1f.ap())
            w2f_sb = P.tile([128, (H // 128) * DF], BF16, tag="w2f")
            nc.sync.dma_start(w2f_sb[:, :], w2f.ap().rearrange(
                "(m p) d -> p m d", p=128))
            b2f8_sb = P.tile([128, NIC * DF], F32, tag="b2f8")
            nc.sync.dma_start(b2f8_sb[:, :], b2f8.ap())
            ident_sb = P.tile([128, 128], BF16, tag="ident")
            nc.sync.dma_start(ident_sb[:, :], ident.ap())
            # cls-side + aux inputs (queued after the fg-critical ones)
            for c in range(C // 128):
                nc.sync.dma_start(w1c_sb[:, c * H:(c + 1) * H],
                                  w1c_r[:, c:c + 1, :])
            b1c_sb = P.tile([128, H // 128], F32, tag="b1c")
            nc.sync.dma_start(b1c_sb[:, :], b1c.ap())
            w2c_sb = P.tile([128, (H // 128) * DC], BF16, tag="w2c")
            nc.sync.dma_start(w2c_sb[:, :], w2c.ap().rearrange(
                "(m p) d -> p m d", p=128))
            b2c8_sb = P.tile([128, NIC * DC], F32, tag="b2c8")
            nc.sync.dma_start(b2c8_sb[:, :], b2c8.ap())
            fgown_sb = P.tile([128, NIC], F32, tag="fgown")
            nc.sync.dma_start(fgown_sb[:, :], fgown.ap())
            iou_sb = P.tile([128, NIC], F32, tag="iou")
            nc.sync.dma_start(iou_sb[:, :], iou.ap())
            ohb_sb = P.tile([128, NIC * NCLS], BF16, tag="ohb")
            nc.sync.dma_start(ohb_sb[:, :], ohb.ap())
            identF_sb = P.tile([128, 128], F32, tag="identF")
            nc.sync.dma_start(identF_sb[:, :], identF.ap())
            cswt_sb = P.tile([128, 4 * 8], BF16, tag="cswt")
            nc.sync.dma_start(cswt_sb[:, :], cswt.ap())
            cswb_sb = P.tile([128, 4 * 8], BF16, tag="cswb")
            nc.sync.dma_start(cswb_sb[:, :], cswb.ap())
            cswp_sb = P.tile([128, 4 * 8], BF16, tag="cswp")
            nc.sync.dma_start(cswp_sb[:, :], cswp.ap())
            cswc_sb = P.tile([128, 4 * 4], BF16, tag="cswc")
            nc.sync.dma_start(cswc_sb[:, :], cswc.ap())
            bgw_sb = P.tile([128, 3 * NIC], F32, tag="bgw")
            nc.sync.dma_start(bgw_sb[:, :], bgw.ap())

            onesR_sb = P.tile([1, 128], F32, tag="onesR")    # outer-product lhsT
            nc.vector.memset(onesR_sb[:, :], 1.0)
            onesP_sb = P.tile([128, 1], F32, tag="onesP")    # final reduce lhsT
            nc.vector.memset(onesP_sb[:, :], 1.0)
            eps2_sb = P.tile([128, 1], F32, tag="eps2")
            nc.vector.memset(eps2_sb[:, :], 2.0 * EPS)
            eps1_sb = P.tile([128, 1], F32, tag="eps1")
            nc.vector.memset(eps1_sb[:, :], EPS)

            # persistent SBUF tiles
            znfT128 = P.tile([128, SH], BF16, tag="znfT128")  # fg zT, K-pad
            nc.vector.memset(znfT128[64:128, :], 0.0)
            znf8 = P.tile([128, SH], F8, tag="znf8")          # fp8 copy
            nc.vector.memset(znf8[64:128, :], 0.0)
            zncT_sb = P.tile([128, SH], BF16, tag="zncT")
            znc8 = P.tile([128, SH], F8, tag="znc8")
            ssqf_sb = P.tile([128, NIC], F32, tag="ssqf")
            ssqc_sb = P.tile([128, NIC], F32, tag="ssqc")
            spos_sb = P.tile([128, NIC], F32, tag="spos")
            npos_sb = P.tile([128, NIC], F32, tag="npos")
            zfT_rem = P.tile([128, KW], F8, tag="zfT_rem")    # K-padded
            nc.vector.memset(zfT_rem[64:128, :], 0.0)
            zcT_rem = P.tile([128, KW], F8, tag="zcT_rem")
            cb_sb = P.tile([NCLS, DC + 1], F32, tag="cb_sb")
            cbl_sb = P.tile([NCLS, DC + 1], F32, tag="cbl_sb")
            zbcT_sb = P.tile([128, NCLS], BF16, tag="zbcT_sb")
            hist_sb = P.tile([1, NCLS], F32, tag="hist_sb")
            fgtot_sb = P.tile([1, 1], F32, tag="fgtot")
            histB_sb = P.tile([128, NCLS], F32, tag="histB")
            ftB_sb = P.tile([128, 1], F32, tag="ftB")
            # accum slots per i-chunk (8 each):
            # fg: 0..2 = k1..k3 tiles, 3 = k4 quarter, 4 = diag,
            #     5 = bg part of slot-0 tile, 6 = bg part of k4-hi,
            #     7 = bg part of diag
            # cls: 0..3 = tiles, 4 = diag
            accF = P.tile([128, NIC * 8], F32, tag="accF")
            nc.vector.memset(accF[:, :], 0.0)
            accC = P.tile([128, NIC * 8], F32, tag="accC")
            nc.vector.memset(accC[:, :], 0.0)
            csf_sb = P.tile([8, SH], F32, tag="csf_sb")       # fg colsum copy
            # cls colsum via elementwise E pre-sum (PE does only 8 small
            # matmuls post-loop instead of 64 in-loop)
            esum = [P.tile([128, SH], BF16, tag=f"esum{k}",
                            name=f"esum{k}") for k in range(4)]
            esumB = [P.tile([128, SH], BF16, tag=f"esumB{k}",
                             name=f"esumB{k}") for k in range(4)]
            esumC = [P.tile([128, SH], BF16, tag=f"esumC{k}",
                             name=f"esumC{k}") for k in range(4)]
            csc_sb = P.tile([4, SH], F32, tag="csc_sb")
            outF_sb = P.tile([128, 64], F32, tag="outF")      # fg outbox
            outC_sb = P.tile([128, 32], F32, tag="outC")

            # ================= fg head pipeline =================
            hTf_sb = P.tile([128, (H // 128) * SH], BF16, tag="hTf")
            PH1ctx = tc.tile_pool(name="ph1", bufs=1, space="PSUM")
            PH1 = PH1ctx.__enter__()
            for m in range(H // 128):           # 2 H-chunks
                pq = [PH1.tile([128, 256], F32, tag=f"hps{q}",
                               name=f"hps{q}", bufs=(2 if q < 3 else 1))
                      for q in range(4)]
                for c in range(C // 128):       # 8 K-chunks
                    for q in range(4):
                        nc.tensor.matmul(
                            pq[q][:, :],
                            lhsT=w1f_sb[:, c * H + m * 128:c * H + (m + 1) * 128],
                            rhs=xT_sb[:, c * SH + q * 256:c * SH + q * 256 + 256],
                            start=(c == 0), stop=(c == C // 128 - 1))
                for q in range(4):
                    nc.vector.tensor_scalar(
                        hTf_sb[:, m * SH + q * 256:m * SH + q * 256 + 256],
                        pq[q][:, :], b1f_sb[:, m:m + 1], 0.0,
                        ALU.add, ALU.max)
            PH1ctx.__exit__(None, None, None)

            PZctx = tc.tile_pool(name="pzf", bufs=1, space="PSUM")
            PZ = PZctx.__enter__()
            PTctx = tc.tile_pool(name="ptrf", bufs=1, space="PSUM")
            PT = PTctx.__enter__()
            zf_ps = PZ.tile([128, NIC * DF], F32, tag="zf")      # 1 bank
            for ic in range(NIC):
                for hm in range(H // 128):
                    nc.tensor.matmul(
                        zf_ps[:, ic * DF:(ic + 1) * DF],
                        lhsT=hTf_sb[:, hm * SH + ic * 128:hm * SH + ic * 128 + 128],
                        rhs=w2f_sb[:, hm * DF:(hm + 1) * DF],
                        start=(hm == 0), stop=(hm == H // 128 - 1))
            ztf = P.tile([128, NIC * DF], F32, tag="ztf")
            nc.vector.tensor_add(ztf[:, :], zf_ps[:, :], b2f8_sb[:, :])
            sqf = W.tile([128, NIC * DF], F32, tag="sqf")
            nc.vector.tensor_mul(sqf[:, :], ztf[:, :], ztf[:, :])
            sqf_v = sqf[:, :].rearrange("p (i c) -> p i c", i=NIC)
            n2f = P.tile([128, NIC], F32, tag="n2f")
            nc.vector.tensor_reduce(n2f[:, :], sqf_v,
                                    mybir.AxisListType.X, ALU.add)
            lnf = P.tile([128, NIC], F32, tag="lnf")
            nc.scalar.activation(lnf[:, :], n2f[:, :], AF.Ln)
            ninvf = P.tile([128, NIC], F32, tag="ninvf")
            nc.scalar.activation(ninvf[:, :], lnf[:, :], AF.Exp, scale=-0.5)
            znf = P.tile([128, NIC * DF], BF16, tag="znf")
            for ic in range(NIC):
                nc.vector.tensor_scalar_mul(
                    znf[:, ic * DF:(ic + 1) * DF],
                    ztf[:, ic * DF:(ic + 1) * DF], ninvf[:, ic:ic + 1])
            sqzf = W.tile([128, NIC * DF], F32, tag="sqzf")
            nc.vector.tensor_mul(sqzf[:, :], znf[:, :], znf[:, :])
            nc.vector.tensor_reduce(
                ssqf_sb[:, :], sqzf[:, :].rearrange("p (i c) -> p i c", i=NIC),
                mybir.AxisListType.X, ALU.add)
            for ic in range(NIC):
                zfT_ps = PT.tile([64, 128], BF16, tag="ztrf",
                                 name="zfT_ps", bufs=2)
                nc.tensor.transpose(zfT_ps[:, :],
                                    znf[:, ic * DF:(ic + 1) * DF],
                                    ident_sb[:, :])
                nc.vector.tensor_copy(znfT128[0:64, ic * 128:(ic + 1) * 128],
                                      zfT_ps[:, :])
            # fg z out + AllGather A (fp8)
            nc.vector.tensor_copy(znf8[0:64, :], znfT128[0:64, :])
            nc.sync.dma_start(zpackF.ap(), znf8[0:64, :])
            ag_f = nc.gpsimd.collective_compute(
                "AllGather", ALU.bypass, replica_groups=rg,
                ins=[zpackF.ap().opt()], outs=[zgathF.ap().opt()])
            PTctx.__exit__(None, None, None)
            PZctx.__exit__(None, None, None)

            # ================= cls head pipeline =================
            hTc_sb = P.tile([128, (H // 128) * SH], BF16, tag="hTc")
            PH2ctx = tc.tile_pool(name="ph2", bufs=1, space="PSUM")
            PH2 = PH2ctx.__enter__()
            for m in range(H // 128):
                pq = [PH2.tile([128, 256], F32, tag=f"cps{q}",
                               name=f"cps{q}", bufs=(2 if q < 3 else 1))
                      for q in range(4)]
                for c in range(C // 128):
                    for q in range(4):
                        nc.tensor.matmul(
                            pq[q][:, :],
                            lhsT=w1c_sb[:, c * H + m * 128:c * H + (m + 1) * 128],
                            rhs=xT_sb[:, c * SH + q * 256:c * SH + q * 256 + 256],
                            start=(c == 0), stop=(c == C // 128 - 1))
                for q in range(4):
                    nc.vector.tensor_scalar(
                        hTc_sb[:, m * SH + q * 256:m * SH + q * 256 + 256],
                        pq[q][:, :], b1c_sb[:, m:m + 1], 0.0,
                        ALU.add, ALU.max)
            PH2ctx.__exit__(None, None, None)

            PZ2ctx = tc.tile_pool(name="pzc", bufs=1, space="PSUM")
            PZ2 = PZ2ctx.__enter__()
            PT2ctx = tc.tile_pool(name="ptrc", bufs=1, space="PSUM")
            PT2 = PT2ctx.__enter__()
            PCctx = tc.tile_pool(name="pcb", bufs=1, space="PSUM")
            PC = PCctx.__enter__()
            zc_ps = PZ2.tile([128, NIC * DC], F32, tag="zc")     # 2 banks
            for ic in range(NIC):
                for hm in range(H // 128):
                    nc.tensor.matmul(
                        zc_ps[:, ic * DC:(ic + 1) * DC],
                        lhsT=hTc_sb[:, hm * SH + ic * 128:hm * SH + ic * 128 + 128],
                        rhs=w2c_sb[:, hm * DC:(hm + 1) * DC],
                        start=(hm == 0), stop=(hm == H // 128 - 1))
            ztc = P.tile([128, NIC * DC], F32, tag="ztc")
            nc.vector.tensor_add(ztc[:, :], zc_ps[:, :], b2c8_sb[:, :])
            sqc = W.tile([128, NIC * DC], F32, tag="sqc")
            nc.vector.tensor_mul(sqc[:, :], ztc[:, :], ztc[:, :])
            n2c = P.tile([128, NIC], F32, tag="n2c")
            nc.vector.tensor_reduce(
                n2c[:, :], sqc[:, :].rearrange("p (i c) -> p i c", i=NIC),
                mybir.AxisListType.X, ALU.add)
            lnc = P.tile([128, NIC], F32, tag="lnc")
            nc.scalar.activation(lnc[:, :], n2c[:, :], AF.Ln)
            ninvc = P.tile([128, NIC], F32, tag="ninvc")
            nc.scalar.activation(ninvc[:, :], lnc[:, :], AF.Exp, scale=-0.5)
            zcat = P.tile([128, NIC * (DC + 1)], BF16, tag="zcat")
            for ic in range(NIC):
                zoff = ic * (DC + 1)
                nc.vector.tensor_scalar_mul(
                    zcat[:, zoff:zoff + DC],
                    ztc[:, ic * DC:(ic + 1) * DC], ninvc[:, ic:ic + 1])
                nc.vector.memset(zcat[:, zoff + DC:zoff + DC + 1], 1.0)
            zc_v = zcat[:, :].rearrange("p (i c) -> p i c", i=NIC)
            sqzc = W.tile([128, NIC * DC], F32, tag="sqzc")
            sqzc_v = sqzc[:, :].rearrange("p (i c) -> p i c", i=NIC)
            nc.vector.tensor_mul(sqzc_v, zc_v[:, :, 0:DC], zc_v[:, :, 0:DC])
            nc.vector.tensor_reduce(ssqc_sb[:, :], sqzc_v,
                                    mybir.AxisListType.X, ALU.add)
            cb_ps = PC.tile([NCLS, DC + 1], F32, tag="cb")
            for ic in range(NIC):
                zoff = ic * (DC + 1)
                nc.tensor.matmul(
                    cb_ps[:, :],
                    lhsT=ohb_sb[:, ic * NCLS:(ic + 1) * NCLS],
                    rhs=zcat[:, zoff:zoff + DC + 1],
                    start=(ic == 0), stop=(ic == NIC - 1))
                zcT_ps = PT2.tile([128, 128], BF16, tag="ztrc",
                                  name="zcT_ps", bufs=2)
                nc.tensor.transpose(zcT_ps[:, :],
                                    zcat[:, zoff:zoff + DC],
                                    ident_sb[:, :])
                nc.vector.tensor_copy(zncT_sb[:, ic * 128:(ic + 1) * 128],
                                      zcT_ps[:, :])
            # cls z out (fp8) + raw f32 class-sum bytes + AllGather B
            nc.vector.tensor_copy(cbl_sb[:, :], cb_ps[:, :])
            nc.vector.tensor_copy(znc8[:, :], zncT_sb[:, :])
            nc.sync.dma_start(zpackC.ap()[0:DC, :], znc8[:, :])
            nc.sync.dma_start(zpackC.ap()[DC:DC + NCLS, 0:CBW],
                              cbl_sb[:, :].bitcast(F8))
            ag_c = nc.gpsimd.collective_compute(
                "AllGather", ALU.bypass, replica_groups=rg,
                ins=[zpackC.ap().opt()], outs=[zgathC.ap().opt()])
            add_dep_helper(ag_c.ins, ag_f.ins,
                           reason="AGf before AGc on cc stream")
            PCctx.__exit__(None, None, None)
            PT2ctx.__exit__(None, None, None)
            PZ2ctx.__exit__(None, None, None)

            # ---- cb-independent precompute (fills AG wait) ----
            edf_sb = P.tile([128, NIC], F32, tag="edf_sb")
            nc.scalar.activation(edf_sb[:, :], ssqf_sb[:, :], AF.Exp,
                                 scale=1.0 / TAU)
            edc_sb = P.tile([128, NIC], F32, tag="edc_sb")
            nc.scalar.activation(edc_sb[:, :], ssqc_sb[:, :], AF.Exp,
                                 scale=1.0 / TAU)
            t0f = P.tile([128, NIC], F32, tag="t0f")
            nc.vector.tensor_mul(t0f[:, :], edf_sb[:, :], fgown_sb[:, :])
            iouw_pre = P.tile([128, NIC], F32, tag="iouw_pre")
            thr0 = W.tile([128, NIC], F32, tag="thr0", name="thr0")
            nc.vector.tensor_scalar(thr0[:, :], iou_sb[:, :], -0.5, 1e9,
                                    ALU.add, ALU.mult)
            nc.vector.tensor_scalar_max(thr0[:, :], thr0[:, :], 0.0)
            nc.vector.tensor_scalar_min(thr0[:, :], thr0[:, :], 1.0)
            nc.vector.tensor_mul(iouw_pre[:, :], iou_sb[:, :], thr0[:, :])

            # ---- remote z extraction (pid-predicated slot rotation) ----
            # slot sources: s0 = 0 if pid>=5 else pid+1; s1/s2 = the other
            # two of {pid+1,pid+2,pid+3}; slot 3 = (pid+4)%8 with the lo
            # half holding column-half (pid>=4) of that block.
            def _slots(pid):
                ge5 = (pid == 5) + (pid == 6) + (pid == 7)
                lt5 = (pid == 0) + (pid == 1) + (pid == 2) + \
                    (pid == 3) + (pid == 4)
                s0 = (pid + 1) * lt5
                s1 = pid + 2 - 8 * ((pid == 6) + (pid == 7)) + (pid == 6) * 7
                s2 = pid + 3 - 8 * ge5 + (pid == 5) * 6
                s3 = (pid + 4) % 8
                ge4 = ge5 + (pid == 4)
                return s0, s1, s2, s3, ge4

            pidF = nc.sync.partition_id()
            sF = _slots(pidF)
            for s in range(2):
                apF = zgathF.ap()[0:DF, :].copy()
                apF.offset = sF[s] * (DF * SH)
                nc.sync.dma_start(zfT_rem[0:64, s * SH:(s + 1) * SH], apF)
            pidS = nc.scalar.partition_id()
            sS = _slots(pidS)
            apF = zgathF.ap()[0:DF, :].copy()
            apF.offset = sS[2] * (DF * SH)
            nc.scalar.dma_start(zfT_rem[0:64, 2 * SH:3 * SH], apF)
            apF = zgathF.ap()[0:DF, 0:512].copy()
            apF.offset = sF[3] * (DF * SH) + sF[4] * 512
            nc.sync.dma_start(zfT_rem[0:64, 3 * SH:3 * SH + 512], apF)
            apF = zgathF.ap()[0:DF, 0:512].copy()
            apF.offset = sS[3] * (DF * SH) + (1 - sS[4]) * 512
            nc.scalar.dma_start(zfT_rem[0:64, 3 * SH + 512:4 * SH], apF)

            pidC = nc.gpsimd.partition_id()
            sC = _slots(pidC)
            for s in range(3):
                apC = zgathC.ap()[0:DC, :].copy()
                apC.offset = sC[s] * (CROWS * SH)
                nc.gpsimd.dma_start(zcT_rem[:, s * SH:(s + 1) * SH], apC)
            apC = zgathC.ap()[0:DC, 0:512].copy()
            apC.offset = sC[3] * (CROWS * SH) + sC[4] * 512
            nc.gpsimd.dma_start(zcT_rem[:, 3 * SH:3 * SH + 512], apC)
            apC = zgathC.ap()[0:DC, 0:512].copy()
            apC.offset = sC[3] * (CROWS * SH) + (1 - sC[4]) * 512
            nc.gpsimd.dma_start(zcT_rem[:, 3 * SH + 512:4 * SH], apC)

            # class-sum partials gather (raw f32 bytes; summed later)
            cbg32 = P.tile([NCLS, NCORES * (DC + 1)], F32, tag="cbg32")
            for r in range(NCORES):
                nc.gpsimd.dma_start(
                    cbg32[:, r * (DC + 1):(r + 1) * (DC + 1)].bitcast(F8),
                    zgathC.ap()[r * CROWS + DC:r * CROWS + DC + NCLS,
                                0:CBW])

            # ================= main loops =================
            PSIMctx = tc.tile_pool(name="psim", bufs=1, space="PSUM")
            PSIM = PSIMctx.__enter__()

            def remote_loop(head, lhsT_all, rem, acc, lw,
                            cs_now=None, hooks=None):
                # jobs: (ic, k).  Colsums happen via elementwise E
                # pre-sums (esum) + a few post-loop matmuls, so the PE
                # only runs sims here and the loop stays ACT-paced.
                jobs = [(ic, k) for ic in range(NIC) for k in range(4)]
                CS = cs_now

                for idx, (ic, k) in enumerate(jobs):
                    lhsT = lhsT_all[:, ic * 128:(ic + 1) * 128]
                    if k == 3:
                        w = 512
                        off = 3 * SH + (0 if ic < 4 else 512)
                    else:
                        w = SH
                        off = k * SH
                    tag = "simD" if k == 3 else "sim"
                    st = PSIM.tile([128, w], F32, tag=tag, name=tag, bufs=2)
                    for q in range(w // 512):
                        nc.tensor.matmul(
                            st[:, q * 512:(q + 1) * 512],
                            lhsT=lhsT,
                            rhs=rem[:, off + q * 512:off + (q + 1) * 512],
                            start=True, stop=True)
                    E = EX.tile([128, SH], BF16, tag=f"E{head}",
                                name=f"E{head}", bufs=6)
                    if k == 3:
                        # k4 rowsum on DVE (keeps the scalar engine free
                        # of the accumulator-read cost)
                        nc.scalar.activation(E[:, 0:w], st[:, 0:w], AF.Exp,
                                             scale=1.0 / TAU)
                        nc.vector.tensor_reduce(
                            acc[:, ic * 8 + 3:ic * 8 + 4],
                            E[:, 0:w].rearrange("p (o c) -> p o c", o=1),
                            mybir.AxisListType.X, ALU.add)
                    else:
                        nc.scalar.activation(
                            E[:, 0:w], st[:, 0:w], AF.Exp, scale=1.0 / TAU,
                            accum_out=acc[:, ic * 8 + k:ic * 8 + k + 1])
                    if n_bg > 0 and head == 'f' and k in (0, 3):
                        slot = 5 if k == 0 else 6
                        nc.vector.tensor_reduce(
                            acc[:, ic * 8 + slot:ic * 8 + slot + 1],
                            E[:, 0:n_bg].rearrange("p (o c) -> p o c", o=1),
                            mybir.AxisListType.X, ALU.add)
                    # pre-sum E tiles elementwise per slot; colsums run
                    # post-loop on the 4 accumulated tiles (PE relief)
                    eo = (0 if ic < 4 else 512) if k == 3 else 0
                    first = (ic == 0) or (k == 3 and ic == 4)
                    eng = nc.gpsimd if ic in (2, 5) else nc.vector
                    es = esum if head == 'f' else esumC
                    if head == 'f' and ic == NBGF and NBGF > 0:
                        # snapshot the bg-prefix partial sum before adding
                        # this chunk (bg rows span chunks [0:NBGF) + a
                        # partial remainder in chunk NBGF)
                        nc.vector.tensor_copy(esumB[k][:, eo:eo + w],
                                              esum[k][:, eo:eo + w])
                    if first:
                        eng.tensor_copy(es[k][:, eo:eo + w], E[:, 0:w])
                    else:
                        eng.tensor_add(es[k][:, eo:eo + w],
                                       es[k][:, eo:eo + w], E[:, 0:w])
                    if head == 'f' and ic == NBGF and NBGR > 0:
                        # partial-chunk bg colsum straight into CS; the
                        # k==0 matmuls also zero the CS region
                        for q in range(w // 512):
                            nc.tensor.matmul(
                                CS[0:lw, eo + q * 512:eo + (q + 1) * 512],
                                lhsT=cswp_sb[:, k * 8:(k + 1) * 8],
                                rhs=E[:, q * 512:q * 512 + 512],
                                start=(k == 0), stop=False,
                                skip_group_check=True)
                    if hooks and idx in hooks:
                        r = hooks[idx]()
                        if r is not None:
                            CS = r

            def diag_loop(head, lhsT_all, acc):
                for ic in range(NIC):
                    lhsT = lhsT_all[:, ic * 128:(ic + 1) * 128]
                    st = PSIM.tile([128, SH], F32, tag="sim", name="sim",
                                   bufs=2)
                    for q in range(SH // 512):
                        nc.tensor.matmul(
                            st[:, q * 512:(q + 1) * 512],
                            lhsT=lhsT,
                            rhs=lhsT_all[:, q * 512:(q + 1) * 512],
                            start=True, stop=True)
                    E = EX.tile([128, SH], BF16, tag=f"E{head}",
                                name=f"E{head}", bufs=6)
                    nc.scalar.activation(
                        E[:, :], st[:, :], AF.Exp, scale=1.0 / TAU,
                        accum_out=acc[:, ic * 8 + 4:ic * 8 + 5])
                    if n_bg > 0 and head == 'f':
                        nc.vector.tensor_reduce(
                            acc[:, ic * 8 + 7:ic * 8 + 8],
                            E[:, 0:n_bg].rearrange("p (o c) -> p o c", o=1),
                            mybir.AxisListType.X, ALU.add)

            # fg diagonal pass first: needs no remote data, so it fills
            # the initial barrier + z-AllGather latency window
            diag_loop('f', znfT128, accF)

            # ---- fg remote loop + colsums ----
            PCSFctx = tc.tile_pool(name="pcsf", bufs=1, space="PSUM")
            PCSF = PCSFctx.__enter__()
            CSF = PCSF.tile([8, SH], F32, tag="CSF")
            remote_loop('f', znf8, zfT_rem, accF, 8, cs_now=CSF)
            # post-loop fg colsums from the pre-summed E tiles
            for k in range(4):
                for q in range(2):
                    nc.tensor.matmul(
                        CSF[0:8, q * 512:(q + 1) * 512],
                        lhsT=cswt_sb[:, k * 8:(k + 1) * 8],
                        rhs=esum[k][:, q * 512:(q + 1) * 512],
                        start=(NBGR == 0 and k == 0),
                        stop=(NBGF == 0 and k == 3 and q == 1),
                        skip_group_check=True)
            if NBGF > 0:
                for k in range(4):
                    qr = 1 if k == 3 else 2
                    for q in range(qr):
                        nc.tensor.matmul(
                            CSF[0:8, q * 512:(q + 1) * 512],
                            lhsT=cswb_sb[:, k * 8:(k + 1) * 8],
                            rhs=esumB[k][:, q * 512:(q + 1) * 512],
                            start=False, stop=(k == 3),
                            skip_group_check=True)

            def fg_finalize():
                # fg colsums -> panels -> outbox -> AGxf; frees the CSF
                # banks and hands back the cls CS tile.  Runs a couple of
                # cls jobs into the loop so the PE/ACT never stall on it.
                nc.vector.tensor_copy(csf_sb[:, :], CSF[:, :])
                PCSFctx.__exit__(None, None, None)
                for j in range(8):
                    tp = PSIM.tile([128, 512], F32, tag="simD", name="simD",
                                   bufs=2)
                    nc.tensor.transpose(tp[:, 0:8],
                                        csf_sb[0:8, j * 128:(j + 1) * 128],
                                        identF_sb[0:8, 0:8])
                    dst = outF_sb[:, :].rearrange("p (k t j) -> p k t j",
                                                  k=4, t=2)[:, :, :, j:j + 1]
                    src = tp[:, 0:8].rearrange("p (k t o) -> p k t o",
                                               k=4, t=2)
                    nc.vector.tensor_copy(dst, src)
                nc.sync.dma_start(oboxF.ap(), outF_sb[:, :])
                ag_xf = nc.gpsimd.collective_compute(
                    "AllGather", ALU.bypass, replica_groups=rg,
                    ins=[oboxF.ap().opt()], outs=[goboxF.ap().opt()])
                add_dep_helper(ag_xf.ins, ag_c.ins,
                               reason="AGc before AGxf on cc stream")
                fg_finalize.ag = ag_xf
                PCSC = fg_finalize.ctx.__enter__()
                fg_finalize.csc = PCSC.tile([4, SH], F32, tag="CSC")
                return fg_finalize.csc

            fg_finalize.ctx = tc.tile_pool(name="pcsc", bufs=1,
                                           space="PSUM")

            def phase4():
                # class-sum tree + zbar/hist/spos/npos; latency-bound
                # chain of small ops, so it runs inside the cls loop's
                # engine slack instead of serializing the tail
                cbs4 = P.tile([NCLS, 4 * (DC + 1)], F32, tag="cbs4")
                nc.vector.tensor_add(cbs4[:, :], cbg32[:, 0:4 * (DC + 1)],
                                     cbg32[:, 4 * (DC + 1):8 * (DC + 1)])
                cbs2 = P.tile([NCLS, 2 * (DC + 1)], F32, tag="cbs2")
                nc.vector.tensor_add(cbs2[:, :], cbs4[:, 0:2 * (DC + 1)],
                                     cbs4[:, 2 * (DC + 1):4 * (DC + 1)])
                nc.vector.tensor_add(cb_sb[:, :], cbs2[:, 0:DC + 1],
                                     cbs2[:, DC + 1:2 * (DC + 1)])

                t1 = PSIM.tile([128, 512], F32, tag="simD", name="simD",
                               bufs=2)
                zbcT_ps = t1[:, 0:NCLS]
                nc.tensor.transpose(zbcT_ps, cb_sb[:, 0:DC],
                                    identF_sb[0:NCLS, 0:NCLS])
                nc.vector.tensor_copy(zbcT_sb[:, :], zbcT_ps)
                hist_ps = t1[0:1, 32:32 + NCLS]
                nc.tensor.transpose(hist_ps, cb_sb[:, DC:DC + 1],
                                    identF_sb[0:NCLS, 0:NCLS])
                nc.vector.tensor_copy(hist_sb[:, :], hist_ps)
                nc.vector.tensor_reduce(fgtot_sb[:, :], hist_sb[:, :],
                                        mybir.AxisListType.X, ALU.add)
                hb_ps = t1[:, 64:64 + NCLS + 1]
                nc.tensor.matmul(hb_ps[:, 0:NCLS], lhsT=onesR_sb[:, :],
                                 rhs=hist_sb[:, :], start=True, stop=True)
                nc.tensor.matmul(hb_ps[:, NCLS:NCLS + 1],
                                 lhsT=onesR_sb[:, :],
                                 rhs=fgtot_sb[:, :], start=True, stop=True)
                nc.vector.tensor_copy(histB_sb[:, :], hb_ps[:, 0:NCLS])
                nc.vector.tensor_copy(ftB_sb[:, :],
                                      hb_ps[:, NCLS:NCLS + 1])

                t2 = PSIM.tile([128, 512], F32, tag="simD", name="simD",
                               bufs=2)
                gall_ps = t2[:, 0:NIC * 32]
                for ic in range(NIC):
                    nc.tensor.matmul(gall_ps[:, ic * 32:ic * 32 + NCLS],
                                     lhsT=zncT_sb[:, ic * 128:
                                                  (ic + 1) * 128],
                                     rhs=zbcT_sb[:, :],
                                     start=True, stop=True)
                g_v = gall_ps.rearrange("p (i c) -> p i c", i=NIC)
                oh_v = ohb_sb[:, :].rearrange("p (i c) -> p i c", i=NIC)
                gm = W.tile([128, NIC * NCLS], F32, tag="gm")
                gm_v = gm[:, :].rearrange("p (i c) -> p i c", i=NIC)
                nc.vector.tensor_mul(gm_v, g_v[:, :, 0:NCLS], oh_v)
                nc.vector.tensor_reduce(spos_sb[:, :], gm_v,
                                        mybir.AxisListType.X, ALU.add)
                hb8 = W.tile([128, NIC * NCLS], F32, tag="hb8")
                for r in range(NIC):
                    nc.vector.tensor_copy(hb8[:, r * NCLS:(r + 1) * NCLS],
                                          histB_sb[:, :])
                nm = W.tile([128, NIC * NCLS], F32, tag="nm")
                nm_v = nm[:, :].rearrange("p (i c) -> p i c", i=NIC)
                nc.vector.tensor_mul(
                    nm_v, hb8[:, :].rearrange("p (i c) -> p i c", i=NIC),
                    oh_v)
                nc.vector.tensor_reduce(npos_sb[:, :], nm_v,
                                        mybir.AxisListType.X, ALU.add)

            # ---- cls remote loop + colsums (fg outbox emitted inside) ----
            remote_loop('c', znc8, zcT_rem, accC, 4,
                        hooks={2: fg_finalize, 12: phase4})
            ag_xf = fg_finalize.ag
            CSC = fg_finalize.csc
            for k in range(4):
                for q in range(2):
                    nc.tensor.matmul(
                        CSC[0:4, q * 512:(q + 1) * 512],
                        lhsT=cswc_sb[:, k * 4:(k + 1) * 4],
                        rhs=esumC[k][:, q * 512:(q + 1) * 512],
                        start=(k == 0), stop=(k == 3),
                        skip_group_check=True)
            nc.vector.tensor_copy(csc_sb[:, :], fg_finalize.csc[:, :])
            fg_finalize.ctx.__exit__(None, None, None)

            for j in range(8):
                tp = PSIM.tile([128, 512], F32, tag="simD", name="simD",
                               bufs=2)
                nc.tensor.transpose(tp[:, 0:4],
                                    csc_sb[0:4, j * 128:(j + 1) * 128],
                                    identF_sb[0:4, 0:4])
                dst = outC_sb[:, :].rearrange("p (k j) -> p k j",
                                              k=4)[:, :, j:j + 1]
                src = tp[:, 0:4].rearrange("p (k o) -> p k o", k=4)
                nc.vector.tensor_copy(dst, src)
            nc.sync.dma_start(oboxC.ap(), outC_sb[:, :])
            ag_xc = nc.gpsimd.collective_compute(
                "AllGather", ALU.bypass, replica_groups=rg,
                ins=[oboxC.ap().opt()], outs=[goboxC.ap().opt()])
            add_dep_helper(ag_xc.ins, ag_xf.ins,
                           reason="AGxf before AGxc on cc stream")

            # cls diagonal pass (overlaps AGxc)
            diag_loop('c', zncT_sb, accC)

            # ---- incoming panels (after AGxf / AGxc) ----
            def _recv(pid):
                ge4R = (pid == 4) + (pid == 5) + (pid == 6) + (pid == 7)
                rks = [
                    ((pid + 7) % 8, (pid == 6) * 2 + (pid == 7)),
                    ((pid + 6) % 8, 1 - (pid == 0)),
                    ((pid + 5) % 8, 2 - (pid == 0) * 2),
                ]
                return ge4R, rks, (pid + 4) % 8

            inF = P.tile([128, 4 * 16], F32, tag="inF")
            inC = P.tile([128, 4 * 8], F32, tag="inC")
            ge4R, rks, s4R = _recv(nc.sync.partition_id())
            for i, (snd, slot) in enumerate(rks):
                apf = goboxF.ap()[0:128, 0:16].copy()
                apf.offset = snd * (128 * 64) + slot * 16
                nc.sync.dma_start(inF[:, i * 16:(i + 1) * 16], apf)
            for half in range(2):
                sw = (1 - ge4R) if half == 0 else ge4R
                apf = goboxF.ap()[0:128, 0:4].copy()
                apf.offset = s4R * (128 * 64) + 48 + sw * 4
                nc.sync.dma_start(inF[:, 48 + half * 4:48 + half * 4 + 4],
                                  apf)
                apf = goboxF.ap()[0:128, 0:4].copy()
                apf.offset = s4R * (128 * 64) + 56 + sw * 4
                nc.sync.dma_start(inF[:, 56 + half * 4:56 + half * 4 + 4],
                                  apf)
            # cls panels split across the idle gpsimd + scalar DMA queues
            # so the post-AGxc tail is as short as possible
            ge4G, rksG, s4G = _recv(nc.gpsimd.partition_id())
            for i, (snd, slot) in enumerate(rksG[0:2]):
                apc = goboxC.ap()[0:128, 0:8].copy()
                apc.offset = snd * (128 * 32) + slot * 8
                nc.gpsimd.dma_start(inC[:, i * 8:(i + 1) * 8], apc)
            ge4S, rksS, s4S = _recv(nc.scalar.partition_id())
            snd, slot = rksS[2]
            apc = goboxC.ap()[0:128, 0:8].copy()
            apc.offset = snd * (128 * 32) + slot * 8
            nc.scalar.dma_start(inC[:, 16:24], apc)
            for half in range(2):
                if half == 0:
                    sw = 1 - ge4S
                    apc = goboxC.ap()[0:128, 0:4].copy()
                    apc.offset = s4S * (128 * 32) + 24 + sw * 4
                    nc.scalar.dma_start(inC[:, 24:28], apc)
                else:
                    sw = ge4G
                    apc = goboxC.ap()[0:128, 0:4].copy()
                    apc.offset = s4G * (128 * 32) + 24 + sw * 4
                    nc.gpsimd.dma_start(inC[:, 28:32], apc)

            # ---- fg final math ----
            accF_v = accF[:, :].rearrange("p (i s) -> p i s", i=NIC)
            inF_v = inF[:, :].rearrange("p (kt j) -> p kt j", kt=8)
            totf = P.tile([128, NIC], F32, tag="totf")
            nc.vector.tensor_reduce(totf[:, :], accF_v[:, :, 0:5],
                                    mybir.AxisListType.X, ALU.add)
            pan = W.tile([128, NIC], F32, tag="pan", name="pan")
            pan_v = pan[:, :].rearrange("p (o j) -> p o j", o=1)
            pan2 = W.tile([128, NIC], F32, tag="pan2", name="pan2")
            pan2_v = pan2[:, :].rearrange("p (o j) -> p o j", o=1)
            nc.vector.tensor_add(pan_v, inF_v[:, 0:1, :], inF_v[:, 2:3, :])
            nc.vector.tensor_add(pan2_v, inF_v[:, 4:5, :], inF_v[:, 6:7, :])
            nc.vector.tensor_add(pan[:, :], pan[:, :], pan2[:, :])
            nc.vector.tensor_add(totf[:, :], totf[:, :], pan[:, :])
            # bg: gated own partials + incoming bg panels
            s5 = W.tile([128, NIC], F32, tag="s5", name="s5")
            nc.vector.tensor_copy(
                s5[:, :].rearrange("p (i o) -> p i o", i=NIC),
                accF_v[:, :, 5:6])
            s6 = W.tile([128, NIC], F32, tag="s6", name="s6")
            nc.vector.tensor_copy(
                s6[:, :].rearrange("p (i o) -> p i o", i=NIC),
                accF_v[:, :, 6:7])
            s7 = W.tile([128, NIC], F32, tag="s7", name="s7")
            nc.vector.tensor_copy(
                s7[:, :].rearrange("p (i o) -> p i o", i=NIC),
                accF_v[:, :, 7:8])
            bgf = P.tile([128, NIC], F32, tag="bgf")
            nc.vector.tensor_mul(bgf[:, :], s5[:, :], bgw_sb[:, 0:NIC])
            bg2 = W.tile([128, NIC], F32, tag="bg2", name="bg2")
            nc.vector.tensor_mul(bg2[:, :], s6[:, :],
                                 bgw_sb[:, NIC:2 * NIC])
            nc.vector.tensor_add(bgf[:, :], bgf[:, :], bg2[:, :])
            nc.vector.tensor_mul(bg2[:, :], s7[:, :],
                                 bgw_sb[:, 2 * NIC:3 * NIC])
            nc.vector.tensor_add(bgf[:, :], bgf[:, :], bg2[:, :])
            nc.vector.tensor_add(pan_v, inF_v[:, 1:2, :], inF_v[:, 3:4, :])
            nc.vector.tensor_add(pan2_v, inF_v[:, 5:6, :], inF_v[:, 7:8, :])
            nc.vector.tensor_add(pan[:, :], pan[:, :], pan2[:, :])
            nc.vector.tensor_add(bgf[:, :], bgf[:, :], pan[:, :])

            denom = P.tile([128, NIC], F32, tag="denom")
            nc.vector.tensor_sub(denom[:, :], totf[:, :], edf_sb[:, :])
            numer = P.tile([128, NIC], F32, tag="numer")
            nc.vector.tensor_sub(numer[:, :], totf[:, :], bgf[:, :])
            nc.vector.tensor_sub(numer[:, :], numer[:, :], t0f[:, :])
            lnd = P.tile([128, NIC], F32, tag="lnd")
            nc.scalar.activation(lnd[:, :], denom[:, :], AF.Ln,
                                 bias=eps2_sb[:, 0:1])
            lnn = P.tile([128, NIC], F32, tag="lnn")
            nc.scalar.activation(lnn[:, :], numer[:, :], AF.Ln,
                                 bias=eps1_sb[:, 0:1])
            lossf = P.tile([128, NIC], F32, tag="lossf")
            nc.vector.tensor_sub(lossf[:, :], lnd[:, :], lnn[:, :])

            # fg weights / validity
            nposf = W.tile([128, NIC], F32, tag="nposf", name="nposf")
            nc.vector.tensor_scalar(nposf[:, :], fgown_sb[:, :], -1.0,
                                    ftB_sb[:, 0:1], ALU.mult, ALU.add)
            vf = W.tile([128, NIC], F32, tag="vf", name="vf")
            nc.vector.tensor_scalar_min(vf[:, :], nposf[:, :], 1.0)
            validf = W.tile([128, NIC], F32, tag="validf", name="validf")
            nc.vector.tensor_mul(validf[:, :], vf[:, :], fgown_sb[:, :])
            FIN = P.tile([128, 32], F32, tag="FIN")
            nc.vector.tensor_mul(FIN[:, 8:16], iouw_pre[:, :], validf[:, :])
            nc.vector.tensor_mul(FIN[:, 0:8], FIN[:, 8:16], lossf[:, :])

            # ---- cls final math ----
            accC_v = accC[:, :].rearrange("p (i s) -> p i s", i=NIC)
            inC_v = inC[:, :].rearrange("p (k j) -> p k j", k=4)
            totc = P.tile([128, NIC], F32, tag="totc")
            nc.vector.tensor_reduce(totc[:, :], accC_v[:, :, 0:5],
                                    mybir.AxisListType.X, ALU.add)
            cpan = W.tile([128, NIC], F32, tag="cpan", name="cpan")
            cpan_v = cpan[:, :].rearrange("p (o j) -> p o j", o=1)
            cpan2 = W.tile([128, NIC], F32, tag="cpan2", name="cpan2")
            cpan2_v = cpan2[:, :].rearrange("p (o j) -> p o j", o=1)
            nc.vector.tensor_add(cpan_v, inC_v[:, 0:1, :], inC_v[:, 1:2, :])
            nc.vector.tensor_add(cpan2_v, inC_v[:, 2:3, :], inC_v[:, 3:4, :])
            nc.vector.tensor_add(cpan[:, :], cpan[:, :], cpan2[:, :])
            nc.vector.tensor_add(totc[:, :], totc[:, :], cpan[:, :])

            vc = W.tile([128, NIC], F32, tag="vc", name="vc")
            nc.vector.tensor_scalar_min(vc[:, :], npos_sb[:, :], 1.0)
            validc = W.tile([128, NIC], F32, tag="validc", name="validc")
            nc.vector.tensor_mul(validc[:, :], vc[:, :], fgown_sb[:, :])
            nc.vector.tensor_mul(FIN[:, 24:32], iouw_pre[:, :], validc[:, :])
            t2m = P.tile([128, NIC], F32, tag="t2m")
            nc.vector.tensor_sub(t2m[:, :], spos_sb[:, :], ssqc_sb[:, :])
            nc.vector.tensor_scalar(t2m[:, :], t2m[:, :], -1.0 / TAU, 1e9,
                                    ALU.mult, ALU.add)
            npm1 = P.tile([128, NIC], F32, tag="npm1s")
            nc.vector.tensor_scalar_add(npm1[:, :], npos_sb[:, :], -1.0)
            hh = W.tile([128, NIC], F32, tag="hh", name="hh")
            nc.vector.tensor_scalar_add(hh[:, :], npos_sb[:, :], EPS)
            rcp_sb = P.tile([128, NIC], F32, tag="rcp_sb")
            nc.vector.reciprocal(rcp_sb[:, :], hh[:, :])

            denc = W.tile([128, NIC], F32, tag="denc", name="denc")
            nc.vector.tensor_sub(denc[:, :], totc[:, :], edc_sb[:, :])
            lndc = W.tile([128, NIC], F32, tag="lndc", name="lndc")
            nc.scalar.activation(lndc[:, :], denc[:, :], AF.Ln)
            t3 = W.tile([128, NIC], F32, tag="t3", name="t3")
            nc.vector.tensor_mul(t3[:, :], npm1[:, :], lndc[:, :])
            g = W.tile([128, NIC], F32, tag="g", name="g")
            nc.vector.tensor_add(g[:, :], t2m[:, :], t3[:, :])
            lzi = W.tile([128, NIC], F32, tag="lzi", name="lzi")
            nc.vector.tensor_mul(lzi[:, :], g[:, :], rcp_sb[:, :])
            nc.vector.tensor_mul(FIN[:, 16:24], FIN[:, 24:32], lzi[:, :])

            fin_ps = PSIM.tile([128, 512], F32, tag="simD", name="simD",
                               bufs=2)
            nc.tensor.matmul(fin_ps[0:1, 0:32], lhsT=onesP_sb[:, :],
                             rhs=FIN[:, :], start=True, stop=True)
            res4 = P.tile([1, 8], F32, tag="res4")
            nc.vector.tensor_reduce(
                res4[:, 0:4],
                fin_ps[0:1, 0:32].rearrange("p (q c) -> p q c", q=4),
                mybir.AxisListType.X, ALU.add)
            nc.vector.tensor_copy(res4[:, 4:5], fgtot_sb[:, :])
            nc.vector.memset(res4[:, 5:8], 0.0)
            nc.sync.dma_start(psums.ap(), res4[:, :])

            PSIMctx.__exit__(None, None, None)

    nc.compile()
    return nc


def _prep_inputs(roi_feats, labels, ious, fg_w1, fg_b1, fg_w2, fg_b2,
                 cls_w1, cls_b1, cls_w2, cls_b2):
    bf = ml_dtypes.bfloat16
    labels = np.asarray(labels).astype(np.int64)
    ious = np.asarray(ious, np.float32)
    roi = np.asarray(roi_feats, np.float32)

    # permute rows: background (label==0) first; loss is invariant
    perm = np.argsort(labels != 0, kind="stable")
    n_bg = int(np.sum(labels == 0))
    roi = roi[perm]
    labels = labels[perm]
    ious = ious[perm]

    b1f = np.ascontiguousarray(
        np.asarray(fg_b1, np.float32).reshape(H // 128, 128).T)
    b1c = np.ascontiguousarray(
        np.asarray(cls_b1, np.float32).reshape(H // 128, 128).T)
    b2f8 = np.tile(np.tile(np.asarray(fg_b2, np.float32), (128, 1)),
                   (1, NIC))
    b2c8 = np.tile(np.tile(np.asarray(cls_b2, np.float32), (128, 1)),
                   (1, NIC))

    fg_glob = (labels > 0).astype(np.float32)
    ident = np.eye(128, dtype=np.float32)

    oh_glob = np.zeros((N, NCLS), np.float32)
    oh_glob[np.arange(N), labels % NCLS] = (labels > 0)

    # cls colsum lhsT: block k has ones in col k
    cswc = np.zeros((128, 4 * 4), np.float32)
    for k in range(4):
        cswc[:, k * 4 + k] = 1.0

    in_maps = []
    for k in range(NCORES):
        sl = slice(k * SH, (k + 1) * SH)
        oh_own = oh_glob[sl]
        ohb = np.concatenate(
            [oh_own[ic * 128:(ic + 1) * 128] for ic in range(NIC)],
            axis=1).astype(bf)
        # fg colsum lhsT variants (block s: tot col 2s / bg col 2s+1)
        cswt = np.zeros((128, 4 * 8), np.float32)
        cswb = np.zeros((128, 4 * 8), np.float32)
        cswp = np.zeros((128, 4 * 8), np.float32)
        for s in range(4):
            cswt[:, s * 8 + 2 * s] = 1.0
            if k == 0:
                cswb[:, s * 8 + 2 * s + 1] = 1.0
                cswp[0:(n_bg % 128), s * 8 + 2 * s + 1] = 1.0
        # bg gates (g-major): slot0-tile | k4-hi | diag
        bgwa = np.zeros((128, 3 * NIC), np.float32)
        if k in (5, 6, 7):
            bgwa[:, 0:NIC] = 1.0                  # slot-0 tile has bg cols
        if k == 4:
            bgwa[:, NIC + 4:NIC + 8] = 1.0        # k4-hi (ic>=4) has bg cols
        if k == 0:
            bgwa[:, 2 * NIC:3 * NIC] = 1.0        # diag has bg cols
        in_maps.append({
            "xT": np.ascontiguousarray(roi[sl].T).astype(bf),
            "w1f": np.asarray(fg_w1).astype(bf),
            "w1c": np.asarray(cls_w1).astype(bf),
            "b1f": b1f,
            "b1c": b1c,
            "w2f": np.asarray(fg_w2).astype(bf),
            "w2c": np.asarray(cls_w2).astype(bf),
            "b2f8": b2f8,
            "b2c8": b2c8,
            "fgown": np.ascontiguousarray(
                fg_glob[sl].reshape(NIC, 128).T).astype(np.float32),
            "iou": np.ascontiguousarray(
                ious[sl].reshape(NIC, 128).T).astype(np.float32),
            "ohb": ohb,
            "ident": ident.astype(bf),
            "identF": ident,
            "cswt": cswt.astype(bf),
            "cswb": cswb.astype(bf),
            "cswp": cswp.astype(bf),
            "cswc": cswc.astype(bf),
            "bgw": bgwa,
        })
    return in_maps, n_bg


def _get_nc(n_bg):
    key = ("nc", n_bg)
    if key not in _cached:
        _cached[key] = _build(n_bg)
    return _cached[key]


def run(inputs, trace=False, tmpdir=None):
    in_maps, n_bg = _prep_inputs(**inputs)
    nc = _get_nc(n_bg)
    res = bass_utils.run_bass_kernel_spmd(
        nc, in_maps, core_ids=list(range(NCORES)), trace=trace, tmpdir=tmpdir)
    swl_f = sw_f = swl_c = sw_c = 0.0
    for r in res.results:
        p = r["psums"][0].astype(np.float64)
        swl_f += p[0]; sw_f += p[1]; swl_c += p[2]; sw_c += p[3]
    loss_fg = swl_f / (sw_f + EPS)
    loss_c = swl_c / (sw_c + EPS)
    out = np.array([loss_fg, loss_c], np.float32)
    return out, res


def kernel(**inputs) -> np.ndarray:
    out, _ = run(inputs)
    return out
